# revision 2
# baseline (speedup 1.0000x reference)
"""Trainium2 Bass kernel for nn_CenterLossN (center-loss style reduction).

Math (per batch n, class c; H=W=384, C=11, N=32):
    res[n,c]   = x[n,c]^2 + centers[n,c]^2 - 2 * x[n,c] @ centers[n,c]
    out[n,h,w] = max_c softmax_c(res)[n,c,h,w] = 1 / sum_c exp(res_c - max_c res_c)
    loss       = sum(clip(out * labels, 1e-12, 1e12)) / (N*H*W)

Device strategy (data-parallel over N across 8 cores, 4 batches/core):
  All matmul inputs in fp8e4m3 with DoubleRow perf mode (0.5 cyc/col).
  Host ships, per batch n, three partition-major images:
    sta [128, 34, 384]: slots 3c+kc = (-2x)^T k-block kc of class c
                        (weights layout [k, h]); slot 33 = I3 = eye(128)
                        tiled 3x along columns.
    mov [128, 66, 384]: slots 6c+kc = centers k-block kc (moving [k, w]);
                        slots 6c+3+mc = ee row-chunk mc where
                        ee = x^2 + centers^2 (moving [h, w]).
    lab [128, 3, 384] bf16: labels row-chunk-major.
  Per (n, c, mc) the 384-deep contraction + ee injection is TWO DoubleRow
  matmuls: DR1 contracts k-blocks {0,1}; DR2 pairs k-block 2 with an
  identity column that injects ee into the same PSUM accumulation
  (weights = [xt2_k2 | I], moving = [cc_k2 | ee_mc] via a strided AP).
  PSUM tiles are 2-class packs [128, 2, 512] f32 (bank-aligned slots) so
  one ACT copy drains two classes f32->bf16. Tail per (n, mc):
  5-op DVE max tree, broadcast-AP subtract (in-place), one batched ACT
  exp, 5-op DVE add tree, bf16 reciprocal, bf16 scalar_tensor_tensor
  with per-chunk f32 accumulate. clip: only label==0 hits the 1e-12
  floor (1/sum is in [1/11, 1]); host adds 1e-12 * count(labels==0).

HW notes from bring-up on this deployment: AluOpType.divide, Ln, ACT
scale!=1.0, tensor_tensor_reduce, custom-DVE and GpSimd tensor ops all
fail; nc.vector.reciprocal works (bf16 in/out verified on HW).
"""

import numpy as np
import ml_dtypes

N, C, H, W = 32, 11, 384, 384
N_CORES = 8
N_LOC = N // N_CORES          # 4 batches per core
MC = H // 128                 # 3 row-chunks
KC = W // 128                 # 3 contraction chunks
NSTA = C * KC + 1             # 34 slots (last = tiled identity)
NMOV = C * 6                  # 66 slots

_BF16 = ml_dtypes.bfloat16
_FP8 = ml_dtypes.float8_e4m3
_COMPILED = None


def _build(n_loc=N_LOC):
    from contextlib import ExitStack
    import concourse.bass as bass
    import concourse.bacc as bacc
    import concourse.tile as tile
    from concourse import mybir

    bf16 = mybir.dt.bfloat16
    f32 = mybir.dt.float32
    fp8 = mybir.dt.float8e4
    AF = mybir.ActivationFunctionType
    DR = mybir.MatmulPerfMode.DoubleRow

    nc = bacc.Bacc("TRN2", target_bir_lowering=False, debug=False)

    sta_d = nc.dram_tensor("sta", [n_loc, 128, NSTA * W], fp8, kind="ExternalInput")
    mov_d = nc.dram_tensor("mov", [n_loc, 128, NMOV * W], fp8, kind="ExternalInput")
    lab_d = nc.dram_tensor("lab", [n_loc, 128, MC * W], bf16, kind="ExternalInput")
    out_d = nc.dram_tensor("out", [128, 1], f32, kind="ExternalOutput")

    with ExitStack() as ctx:
        tc = ctx.enter_context(tile.TileContext(nc))
        loads = ctx.enter_context(tc.tile_pool(name="loads", bufs=2))
        spool = ctx.enter_context(tc.tile_pool(name="spool", bufs=2))
        tree = ctx.enter_context(tc.tile_pool(name="tree", bufs=2))
        small = ctx.enter_context(tc.tile_pool(name="small", bufs=4))
        singles = ctx.enter_context(tc.tile_pool(name="singles", bufs=1))
        pspair = ctx.enter_context(tc.tile_pool(name="pspair", bufs=3, space="PSUM"))
        psone = ctx.enter_context(tc.tile_pool(name="psone", bufs=2, space="PSUM"))

        partial = singles.tile([128, n_loc * MC], f32)

        for n in range(n_loc):
            sta_t = loads.tile([128, NSTA, W], fp8, tag="sta", name=f"sta_{n}")
            nc.sync.dma_start(
                sta_t[:], sta_d[n].rearrange("p (s w) -> p s w", s=NSTA)
            )
            mov_t = loads.tile([128, NMOV, W], fp8, tag="mov", name=f"mov_{n}")
            nc.sync.dma_start(
                mov_t[:], mov_d[n].rearrange("p (s w) -> p s w", s=NMOV)
            )
            lab_t = loads.tile([128, MC, W], bf16, tag="lab", name=f"lab_{n}")
            nc.gpsimd.dma_start(
                lab_t[:], lab_d[n].rearrange("p (s w) -> p s w", s=MC)
            )

            for mc in range(MC):
                S = spool.tile([128, C, W], bf16, tag="S", name=f"S_{n}_{mc}")

                def class_mms(c, out_ap):
                    # DR1: contract k-blocks 0,1 of class c.
                    nc.tensor.matmul(
                        out_ap,
                        sta_t[:, 3 * c : 3 * c + 2, mc * 128 : (mc + 1) * 128],
                        mov_t[:, 6 * c : 6 * c + 2, :],
                        start=True, stop=False, perf_mode=DR,
                    )
                    # DR2: k-block 2 paired with identity -> += ee_mc.
                    wk = sta_t[:, 3 * c + 2, mc * 128 : (mc + 1) * 128]
                    w_ap = bass.AP(
                        tensor=wk.tensor, offset=wk.offset,
                        ap=[list(wk.ap[0]), [(NSTA - 1 - (3 * c + 2)) * W, 2],
                            list(wk.ap[1])],
                    )
                    mv = mov_t[:, 6 * c + 2, :]
                    m_ap = bass.AP(
                        tensor=mv.tensor, offset=mv.offset,
                        ap=[list(mv.ap[0]), [(1 + mc) * W, 2], list(mv.ap[1])],
                    )
                    nc.tensor.matmul(
                        out_ap, w_ap, m_ap, start=False, stop=True, perf_mode=DR,
                    )

                for pair in range(5):
                    pp = pspair.tile([128, 2, 512], f32, tag="pp",
                                     name=f"pp_{n}_{mc}_{pair}")
                    class_mms(2 * pair, pp[:, 0, 0:W])
                    class_mms(2 * pair + 1, pp[:, 1, 0:W])
                    nc.scalar.copy(S[:, 2 * pair : 2 * pair + 2, :], pp[:, :, 0:W])
                ps1 = psone.tile([128, 512], f32, tag="p1", name=f"p1_{n}_{mc}")
                class_mms(10, ps1[0:128, 0:W])
                nc.scalar.copy(S[:, 10, :], ps1[0:128, 0:W])

                # running max over classes: 5-way tree
                m5 = tree.tile([128, 5, W], bf16, tag="m5", name=f"m5_{n}_{mc}")
                nc.vector.tensor_max(m5[:], S[:, 0:5, :], S[:, 5:10, :])
                m2 = tree.tile([128, 2, W], bf16, tag="m2", name=f"m2_{n}_{mc}")
                nc.vector.tensor_max(m2[:], m5[:, 0:2, :], m5[:, 2:4, :])
                m = small.tile([128, W], bf16, tag="m", name=f"m_{n}_{mc}")
                nc.vector.tensor_max(m[:], m2[:, 0, :], m2[:, 1, :])
                nc.vector.tensor_max(m[:], m[:], m5[:, 4, :])
                nc.vector.tensor_max(m[:], m[:], S[:, 10, :])

                # d = s - m in ONE in-place op via step-0 broadcast AP
                m_ap = m[:]
                m_b = bass.AP(
                    tensor=m_ap.tensor, offset=m_ap.offset,
                    ap=[list(m_ap.ap[0]), [0, C], list(m_ap.ap[1])],
                )
                nc.vector.tensor_sub(S[:], S[:], m_b)
                nc.scalar.activation(S[:], S[:], AF.Exp)

                # acc = sum_c e : 5-way tree
                a5 = tree.tile([128, 5, W], bf16, tag="a5", name=f"a5_{n}_{mc}")
                nc.vector.tensor_add(a5[:], S[:, 0:5, :], S[:, 5:10, :])
                a2 = tree.tile([128, 2, W], bf16, tag="a2", name=f"a2_{n}_{mc}")
                nc.vector.tensor_add(a2[:], a5[:, 0:2, :], a5[:, 2:4, :])
                acc = small.tile([128, W], bf16, tag="acc", name=f"acc_{n}_{mc}")
                nc.vector.tensor_add(acc[:], a2[:, 0, :], a2[:, 1, :])
                nc.vector.tensor_add(acc[:], acc[:], a5[:, 4, :])
                nc.vector.tensor_add(acc[:], acc[:], S[:, 10, :])

                t = small.tile([128, W], bf16, tag="t", name=f"t_{n}_{mc}")
                with nc.allow_low_precision(reason="recip feeds 4.7M-elem mean"):
                    nc.vector.reciprocal(t[:], acc[:])
                w_t = small.tile([128, W], bf16, tag="w", name=f"w_{n}_{mc}")
                slot = n * MC + mc
                nc.vector.scalar_tensor_tensor(
                    out=w_t[:], in0=lab_t[:, mc, :], scalar=0.0, in1=t[:],
                    op0=mybir.AluOpType.add, op1=mybir.AluOpType.mult,
                    accum_out=partial[:, slot : slot + 1],
                )

        pf = singles.tile([128, 1], f32)
        nc.vector.tensor_reduce(
            pf[:], partial[:], axis=mybir.AxisListType.X, op=mybir.AluOpType.add
        )
        nc.sync.dma_start(out_d[:, :], pf[:])

    nc.compile()
    return nc


def _get_compiled():
    global _COMPILED
    if _COMPILED is None:
        _COMPILED = _build()
    return _COMPILED


def _host_prep(x, centers, labels):
    x = np.asarray(x, dtype=np.float32)
    centers = np.asarray(centers, dtype=np.float32)
    labels_np = np.asarray(labels)

    n_zero = int((labels_np == 0).sum())

    # sta: [N, 128, NSTA, W]; slot 3c+kc at [p, h] = -2*x[n, c, h, kc*128+p]
    xt2 = np.transpose(x, (0, 1, 3, 2)) * -2.0          # [N, C, W(k), H]
    xt2 = xt2.reshape(N, C, KC, 128, H).astype(_FP8)
    sta = np.empty((N, 128, NSTA, W), dtype=_FP8)
    # [N, C, KC, 128, H] -> [N, 128, C, KC, H]
    sta[:, :, : C * KC, :] = np.transpose(xt2, (0, 3, 1, 2, 4)).reshape(
        N, 128, C * KC, H
    )
    sta[:, :, C * KC, :] = np.tile(np.eye(128, dtype=_FP8), (1, KC))[None]

    # mov: slots 6c+kc = centers k-block; 6c+3+mc = ee row-chunk
    ee = (x * x + centers * centers).astype(_FP8)
    cc8 = centers.astype(_FP8)
    movp = np.empty((N, C, 6, 128, W), dtype=_FP8)
    movp[:, :, 0:3] = cc8.reshape(N, C, KC, 128, W)
    movp[:, :, 3:6] = ee.reshape(N, C, MC, 128, W)
    mov = np.ascontiguousarray(
        np.transpose(movp, (0, 3, 1, 2, 4)).reshape(N, 128, NMOV * W)
    )

    lab = np.ascontiguousarray(
        np.transpose(
            labels_np.astype(np.float32).reshape(N, MC, 128, W), (0, 2, 1, 3)
        ).reshape(N, 128, MC * W)
    ).astype(_BF16)

    sta = sta.reshape(N, 128, NSTA * W)

    in_maps = []
    for core in range(N_CORES):
        sl = slice(core * N_LOC, (core + 1) * N_LOC)
        in_maps.append(
            {
                "sta": np.ascontiguousarray(sta[sl]),
                "mov": np.ascontiguousarray(mov[sl]),
                "lab": np.ascontiguousarray(lab[sl]),
            }
        )
    return in_maps, n_zero


def kernel(x, centers, labels, _trace=False, _trace_kwargs=None):
    from concourse import bass_utils

    nc = _get_compiled()
    in_maps, n_zero = _host_prep(x, centers, labels)

    kwargs = {}
    if _trace:
        kwargs = dict(trace=True, **(_trace_kwargs or {}))
    res = bass_utils.run_bass_kernel_spmd(
        nc, in_maps, core_ids=list(range(N_CORES)), **kwargs
    )

    total = 0.0
    for core in range(N_CORES):
        total += float(res.results[core]["out"].astype(np.float64).sum())
    loss = (total + 1e-12 * n_zero) / float(N * H * W)
    out = np.float32(loss)
    if _trace:
        return out, res
    return out


# revision 4
# speedup vs baseline: 1.2151x; 1.2151x over previous
"""Trainium2 Bass kernel for nn_CenterLossN (center-loss style reduction).

Math (per batch n, class c; H=W=384, C=11, N=32):
    res[n,c]   = x[n,c]^2 + centers[n,c]^2 - 2 * x[n,c] @ centers[n,c]
    out[n,h,w] = max_c softmax_c(res)[n,c,h,w] = 1 / sum_c exp(res_c - max_c res_c)
    loss       = sum(clip(out * labels, 1e-12, 1e12)) / (N*H*W)

Device strategy (data-parallel over N across 8 cores, 4 batches/core):
  All matmul inputs fp8e4m3 with DoubleRow perf mode (0.5 cyc/col).
  Host ships, per batch n, partition-major images:
    sta [128, 34, 384]: slots 3c+kc = (-2x)^T k-block kc of class c
                        (weights [k, h]); slot 33 = eye(128) tiled 3x.
    mov [128, 66, 384]: slots 6c+kc = centers k-block (moving [k, w]);
                        slots 6c+3+mc = ee row-chunk mc (ee = x^2 + c^2).
    lab [128, 3, 384] bf16.
  Per (n, c, mc): DR1 contracts k-blocks {0,1}; DR2 pairs k-block 2 with
  an identity column injecting ee into the same PSUM group (strided APs).
  PSUM tiles are 2-class packs [128, 2, 512] f32 so one ACT copy drains
  two classes f32->bf16.

  The per-(n,mc) tail is software-pipelined 4 deep across chunk
  iterations so every op's inputs are produced in an EARLIER iteration
  (no intra-iteration cross-engine stalls):
    iter i: ACT exp(i-2) | PE matmuls(i) + ACT drains(i)
          | DVE max+sub(i-1) | DVE sum+recip+acc(i-3)
  Reciprocal = u16 bit-trick seed (magic - bits) + one Newton step,
  fused so the final scalar_tensor_tensor's scalar (-1.0008) undoes the
  Newton sign and corrects the seed's -8e-4 mean bias.
  clip: only label==0 hits the 1e-12 floor (1/sum in [1/11, 1]); host
  adds 1e-12 * count(labels==0).

HW notes from bring-up on this deployment: AluOpType.divide, Ln, ACT
scale!=1.0, Reciprocal-on-ACT (table reload per op), tensor_tensor_reduce,
custom-DVE and GpSimd tensor ops all fail or are too slow; InstReciprocal
runs at ~6.6 ns/elem regardless of dtype (no fast mode).
"""

import numpy as np
import ml_dtypes

N, C, H, W = 32, 11, 384, 384
N_CORES = 8
N_LOC = N // N_CORES          # 4 batches per core
MC = H // 128                 # 3 row-chunks
KC = W // 128                 # 3 contraction chunks
NSTA = C * KC + 1             # 34 slots (last = tiled identity)
NMOV = C * 6                  # 66 slots
NCH = N_LOC * MC              # 12 chunks per core
MAGIC = 0x7EF3                # bf16 reciprocal seed constant
BIAS_FIX = -1.0008            # Newton sign + seed-bias correction

_BF16 = ml_dtypes.bfloat16
_FP8 = ml_dtypes.float8_e4m3
_COMPILED = None


def _build(n_loc=N_LOC):
    from contextlib import ExitStack
    import concourse.bass as bass
    import concourse.bacc as bacc
    import concourse.tile as tile
    from concourse import mybir

    bf16 = mybir.dt.bfloat16
    f32 = mybir.dt.float32
    fp8 = mybir.dt.float8e4
    u16 = mybir.dt.uint16
    AF = mybir.ActivationFunctionType
    ALU = mybir.AluOpType
    DR = mybir.MatmulPerfMode.DoubleRow

    nc = bacc.Bacc("TRN2", target_bir_lowering=False, debug=False)

    sta_d = nc.dram_tensor("sta", [n_loc, 128, NSTA * W], fp8, kind="ExternalInput")
    mov_d = nc.dram_tensor("mov", [n_loc, 128, NMOV * W], fp8, kind="ExternalInput")
    lab_d = nc.dram_tensor("lab", [n_loc, 128, MC * W], bf16, kind="ExternalInput")
    cst_d = nc.dram_tensor("cst", [128, W], u16, kind="ExternalInput")
    out_d = nc.dram_tensor("out", [128, 1], f32, kind="ExternalOutput")

    with ExitStack() as ctx:
        tc = ctx.enter_context(tile.TileContext(nc))
        loads = ctx.enter_context(tc.tile_pool(name="loads", bufs=2))
        spool = ctx.enter_context(tc.tile_pool(name="spool", bufs=3))
        dpool = ctx.enter_context(tc.tile_pool(name="dpool", bufs=4))
        tree = ctx.enter_context(tc.tile_pool(name="tree", bufs=2))
        small = ctx.enter_context(tc.tile_pool(name="small", bufs=4))
        singles = ctx.enter_context(tc.tile_pool(name="singles", bufs=1))
        pspair = ctx.enter_context(tc.tile_pool(name="pspair", bufs=3, space="PSUM"))
        psone = ctx.enter_context(tc.tile_pool(name="psone", bufs=2, space="PSUM"))

        partial = singles.tile([128, NCH], f32)
        magic_t = singles.tile([128, W], u16)
        nc.sync.dma_start(magic_t[:], cst_d[:, :])

        sta_ts, mov_ts, lab_ts = {}, {}, {}

        def load_n(n):
            sta_ts[n] = loads.tile([128, NSTA, W], fp8, tag="sta", name=f"sta_{n}")
            nc.sync.dma_start(
                sta_ts[n][:], sta_d[n].rearrange("p (s w) -> p s w", s=NSTA)
            )
            mov_ts[n] = loads.tile([128, NMOV, W], fp8, tag="mov", name=f"mov_{n}")
            nc.sync.dma_start(
                mov_ts[n][:], mov_d[n].rearrange("p (s w) -> p s w", s=NMOV)
            )
            lab_ts[n] = loads.tile([128, MC, W], bf16, tag="lab", name=f"lab_{n}")
            nc.gpsimd.dma_start(
                lab_ts[n][:], lab_d[n].rearrange("p (s w) -> p s w", s=MC)
            )

        load_n(0)

        S_t, D_t = {}, {}
        m_t, acc_t = {}, {}
        r0_t, p_t, u_t = {}, {}, {}

        def stage_mm(i):
            n, mc = i // MC, i % MC
            sta_t, mov_t = sta_ts[n], mov_ts[n]
            S = spool.tile([128, C, W], bf16, tag="S", name=f"S_{i}")
            S_t[i] = S

            def class_mms(c, out_ap):
                nc.tensor.matmul(
                    out_ap,
                    sta_t[:, 3 * c : 3 * c + 2, mc * 128 : (mc + 1) * 128],
                    mov_t[:, 6 * c : 6 * c + 2, :],
                    start=True, stop=False, perf_mode=DR,
                )
                wk = sta_t[:, 3 * c + 2, mc * 128 : (mc + 1) * 128]
                w_ap = bass.AP(
                    tensor=wk.tensor, offset=wk.offset,
                    ap=[list(wk.ap[0]), [(NSTA - 1 - (3 * c + 2)) * W, 2],
                        list(wk.ap[1])],
                )
                mv = mov_t[:, 6 * c + 2, :]
                m_ap = bass.AP(
                    tensor=mv.tensor, offset=mv.offset,
                    ap=[list(mv.ap[0]), [(1 + mc) * W, 2], list(mv.ap[1])],
                )
                nc.tensor.matmul(
                    out_ap, w_ap, m_ap, start=False, stop=True, perf_mode=DR,
                )

            for pair in range(5):
                pp = pspair.tile([128, 2, 512], f32, tag="pp", name=f"pp_{i}_{pair}")
                class_mms(2 * pair, pp[:, 0, 0:W])
                class_mms(2 * pair + 1, pp[:, 1, 0:W])
                nc.scalar.copy(S[:, 2 * pair : 2 * pair + 2, :], pp[:, :, 0:W])
            ps1 = psone.tile([128, 512], f32, tag="p1", name=f"p1_{i}")
            class_mms(10, ps1[0:128, 0:W])
            nc.scalar.copy(S[:, 10, :], ps1[0:128, 0:W])

        def stage_maxsub(i):
            S = S_t[i]
            m5 = tree.tile([128, 5, W], bf16, tag="m5", name=f"m5_{i}")
            nc.vector.tensor_max(m5[:], S[:, 0:5, :], S[:, 5:10, :])
            m2 = tree.tile([128, 2, W], bf16, tag="m2", name=f"m2_{i}")
            nc.vector.tensor_max(m2[:], m5[:, 0:2, :], m5[:, 2:4, :])
            m = small.tile([128, W], bf16, tag="m", name=f"m_{i}")
            nc.vector.tensor_max(m[:], m2[:, 0, :], m2[:, 1, :])
            nc.vector.tensor_max(m[:], m[:], m5[:, 4, :])
            nc.vector.tensor_max(m[:], m[:], S[:, 10, :])

            D = dpool.tile([128, C, W], bf16, tag="D", name=f"D_{i}")
            D_t[i] = D
            m_ap = m[:]
            m_b = bass.AP(
                tensor=m_ap.tensor, offset=m_ap.offset,
                ap=[list(m_ap.ap[0]), [0, C], list(m_ap.ap[1])],
            )
            nc.vector.tensor_sub(D[:], S[:], m_b)

        def stage_exp(i):
            nc.scalar.activation(D_t[i][:], D_t[i][:], AF.Exp)

        def stage_sum(i):
            n, mc = i // MC, i % MC
            D = D_t[i]
            a5 = tree.tile([128, 5, W], bf16, tag="a5", name=f"a5_{i}")
            nc.vector.tensor_add(a5[:], D[:, 0:5, :], D[:, 5:10, :])
            a2 = tree.tile([128, 2, W], bf16, tag="a2", name=f"a2_{i}")
            nc.vector.tensor_add(a2[:], a5[:, 0:2, :], a5[:, 2:4, :])
            acc = small.tile([128, W], bf16, tag="acc", name=f"acc_{i}")
            nc.vector.tensor_add(acc[:], a2[:, 0, :], a2[:, 1, :])
            nc.vector.tensor_add(acc[:], acc[:], a5[:, 4, :])
            nc.vector.tensor_add(acc[:], acc[:], D[:, 10, :])

            # reciprocal: u16 bit-trick seed + one Newton step.
            r0 = small.tile([128, W], bf16, tag="r0", name=f"r0_{i}")
            nc.vector.tensor_sub(
                r0[:].bitcast(u16), magic_t[:], acc[:].bitcast(u16)
            )
            p = small.tile([128, W], bf16, tag="p", name=f"p_{i}")
            nc.vector.tensor_mul(p[:], acc[:], r0[:])
            u = small.tile([128, W], bf16, tag="u", name=f"u_{i}")
            nc.vector.scalar_tensor_tensor(
                out=u[:], in0=p[:], scalar=2.0, in1=r0[:],
                op0=ALU.subtract, op1=ALU.mult,
            )
            w_t = small.tile([128, W], bf16, tag="w", name=f"w_{i}")
            nc.vector.scalar_tensor_tensor(
                out=w_t[:], in0=lab_ts[n][:, mc, :], scalar=BIAS_FIX,
                op0=ALU.mult, op1=ALU.mult, in1=u[:],
                accum_out=partial[:, i : i + 1],
            )

        for i in range(NCH + 3):
            if 0 <= i - 2 < NCH:
                stage_exp(i - 2)
            if i < NCH:
                if i % MC == 1 and (i // MC) + 1 < n_loc:
                    load_n((i // MC) + 1)
                stage_mm(i)
            if 0 <= i - 1 < NCH:
                stage_maxsub(i - 1)
            if 0 <= i - 3 < NCH:
                stage_sum(i - 3)

        pf = singles.tile([128, 1], f32)
        nc.vector.tensor_reduce(
            pf[:], partial[:], axis=mybir.AxisListType.X, op=ALU.add
        )
        nc.sync.dma_start(out_d[:, :], pf[:])

    nc.compile()
    return nc


def _get_compiled():
    global _COMPILED
    if _COMPILED is None:
        _COMPILED = _build()
    return _COMPILED


def _host_prep(x, centers, labels):
    x = np.asarray(x, dtype=np.float32)
    centers = np.asarray(centers, dtype=np.float32)
    labels_np = np.asarray(labels)

    n_zero = int((labels_np == 0).sum())

    # sta: [N, 128, NSTA, W]; slot 3c+kc at [p, h] = -2*x[n, c, h, kc*128+p]
    xt2 = np.transpose(x, (0, 1, 3, 2)) * -2.0          # [N, C, W(k), H]
    xt2 = xt2.reshape(N, C, KC, 128, H).astype(_FP8)
    sta = np.empty((N, 128, NSTA, W), dtype=_FP8)
    sta[:, :, : C * KC, :] = np.transpose(xt2, (0, 3, 1, 2, 4)).reshape(
        N, 128, C * KC, H
    )
    sta[:, :, C * KC, :] = np.tile(np.eye(128, dtype=_FP8), (1, KC))[None]

    # mov: slots 6c+kc = centers k-block; 6c+3+mc = ee row-chunk
    ee = (x * x + centers * centers).astype(_FP8)
    cc8 = centers.astype(_FP8)
    movp = np.empty((N, C, 6, 128, W), dtype=_FP8)
    movp[:, :, 0:3] = cc8.reshape(N, C, KC, 128, W)
    movp[:, :, 3:6] = ee.reshape(N, C, MC, 128, W)
    mov = np.ascontiguousarray(
        np.transpose(movp, (0, 3, 1, 2, 4)).reshape(N, 128, NMOV * W)
    )

    lab = np.ascontiguousarray(
        np.transpose(
            labels_np.astype(np.float32).reshape(N, MC, 128, W), (0, 2, 1, 3)
        ).reshape(N, 128, MC * W)
    ).astype(_BF16)

    sta = sta.reshape(N, 128, NSTA * W)
    cst = np.full((128, W), MAGIC, dtype=np.uint16)

    in_maps = []
    for core in range(N_CORES):
        sl = slice(core * N_LOC, (core + 1) * N_LOC)
        in_maps.append(
            {
                "sta": np.ascontiguousarray(sta[sl]),
                "mov": np.ascontiguousarray(mov[sl]),
                "lab": np.ascontiguousarray(lab[sl]),
                "cst": cst,
            }
        )
    return in_maps, n_zero


def kernel(x, centers, labels, _trace=False, _trace_kwargs=None):
    from concourse import bass_utils

    nc = _get_compiled()
    in_maps, n_zero = _host_prep(x, centers, labels)

    kwargs = {}
    if _trace:
        kwargs = dict(trace=True, **(_trace_kwargs or {}))
    res = bass_utils.run_bass_kernel_spmd(
        nc, in_maps, core_ids=list(range(N_CORES)), **kwargs
    )

    total = 0.0
    for core in range(N_CORES):
        total += float(res.results[core]["out"].astype(np.float64).sum())
    loss = (total + 1e-12 * n_zero) / float(N * H * W)
    out = np.float32(loss)
    if _trace:
        return out, res
    return out


# revision 7
# speedup vs baseline: 1.2687x; 1.0441x over previous
"""Trainium2 Bass kernel for nn_CenterLossN (center-loss style reduction).

Math (per batch n, class c; H=W=384, C=11, N=32):
    res[n,c]   = x[n,c]^2 + centers[n,c]^2 - 2 * x[n,c] @ centers[n,c]
    out[n,h,w] = max_c softmax_c(res)[n,c,h,w] = 1 / sum_c exp(res_c - max_c res_c)
    loss       = sum(clip(out * labels, 1e-12, 1e12)) / (N*H*W)

Device strategy (data-parallel over N across 8 cores, 4 batches/core):
  All matmul inputs fp8e4m3 with DoubleRow perf mode (0.5 cyc/col).
  Host ships, per batch n, partition-major images:
    sta[mc] [128, 34, 128]: slots 3c+kc = (-2x)^T k-block kc of class c,
        columns restricted to row-chunk mc (weights [k, h]); slot 33 =
        eye(128). Split per-mc so the first chunk's weights arrive fast.
    mov_a [128, 44, 384]: per class [cc_k0, cc_k1, cc_k2, ee_mc0].
    mov_b [128, 33, 384]: per class [cc_k2(dup), ee_mc1, ee_mc2].
        (cc = centers moving [k, w]; ee = x^2 + c^2 moving [h, w]; the
        k2 duplicate keeps the DoubleRow (cc_k2, ee_mc) ifmap pair
        inside one tile for mc=1,2.)
    lab [128, 3, 384] bf16.
  Per (n, c, mc): DR1 contracts k-blocks {0,1}; DR2 pairs k-block 2 with
  an identity column injecting ee into the same PSUM group.
  PSUM: two 3-class packs + one 2-class pack per chunk ([128, k, 512]
  f32, bank-aligned slots); one ACT copy drains each pack f32->bf16.

  The per-(n,mc) tail is software-pipelined 4 deep across chunk
  iterations so every op's inputs are produced in an EARLIER iteration:
    iter i: ACT exp(i-2) | PE matmuls(i) + ACT drains(i)
          | DVE max+sub(i-1) | DVE sum+recip+acc(i-3)
  Reciprocal = u16 bit-trick seed (0x7EF3 - bits) + one Newton step;
  the final scalar_tensor_tensor's scalar (-1.003458) undoes the Newton
  sign and corrects the seed's mean bias (value cross-checked against a
  full numpy simulation of the pipeline: -1.003501).
  clip: only label==0 hits the 1e-12 floor (1/sum in [1/11, 1]); host
  adds 1e-12 * count(labels==0).

HW notes from bring-up on this deployment: AluOpType.divide, Ln, ACT
scale!=1.0, Reciprocal-on-ACT (table reload per op), tensor_tensor_reduce,
custom-DVE and GpSimd tensor ops all fail or are too slow; InstReciprocal
runs at ~6.6 ns/elem regardless of dtype (no fast mode).
"""

import numpy as np
import ml_dtypes

N, C, H, W = 32, 11, 384, 384
N_CORES = 8
N_LOC = N // N_CORES          # 4 batches per core
MC = H // 128                 # 3 row-chunks
KC = W // 128                 # 3 contraction chunks
NSTA = C * KC + 1             # 34 weight slots per mc (last = identity)
NMOVA = C * 4                 # 44 slots: cc_k0,cc_k1,cc_k2,ee0 per class
NMOVB = C * 3                 # 33 slots: cc_k2,ee1,ee2 per class
NCH = N_LOC * MC              # 12 chunks per core
MAGIC = 0x7EF3                # bf16 reciprocal seed constant
BIAS_FIX = -1.003458          # Newton sign + seed-bias correction

_BF16 = ml_dtypes.bfloat16
_FP8 = ml_dtypes.float8_e4m3
_COMPILED = None


def _build(n_loc=N_LOC):
    from contextlib import ExitStack
    import concourse.bass as bass
    import concourse.bacc as bacc
    import concourse.tile as tile
    from concourse import mybir

    bf16 = mybir.dt.bfloat16
    f32 = mybir.dt.float32
    fp8 = mybir.dt.float8e4
    u16 = mybir.dt.uint16
    AF = mybir.ActivationFunctionType
    ALU = mybir.AluOpType
    DR = mybir.MatmulPerfMode.DoubleRow

    nc = bacc.Bacc("TRN2", target_bir_lowering=False, debug=False)

    sta_d = nc.dram_tensor("sta", [n_loc, MC, 128, NSTA * 128], fp8,
                           kind="ExternalInput")
    mva_d = nc.dram_tensor("mva", [n_loc, 128, NMOVA * W], fp8,
                           kind="ExternalInput")
    mvb_d = nc.dram_tensor("mvb", [n_loc, 128, NMOVB * W], fp8,
                           kind="ExternalInput")
    lab_d = nc.dram_tensor("lab", [n_loc, 128, MC * W], bf16, kind="ExternalInput")
    cst_d = nc.dram_tensor("cst", [128, W], u16, kind="ExternalInput")
    out_d = nc.dram_tensor("out", [128, 1], f32, kind="ExternalOutput")

    with ExitStack() as ctx:
        tc = ctx.enter_context(tile.TileContext(nc))
        loads = ctx.enter_context(tc.tile_pool(name="loads", bufs=2))
        spool = ctx.enter_context(tc.tile_pool(name="spool", bufs=3))
        dpool = ctx.enter_context(tc.tile_pool(name="dpool", bufs=4))
        tree = ctx.enter_context(tc.tile_pool(name="tree", bufs=2))
        small = ctx.enter_context(tc.tile_pool(name="small", bufs=4))
        singles = ctx.enter_context(tc.tile_pool(name="singles", bufs=1))
        ps3 = ctx.enter_context(tc.tile_pool(name="ps3", bufs=2, space="PSUM"))
        ps2 = ctx.enter_context(tc.tile_pool(name="ps2", bufs=1, space="PSUM"))

        partial = singles.tile([128, NCH], f32)
        magic_t = singles.tile([128, W], u16)
        nc.sync.dma_start(magic_t[:], cst_d[:, :])

        sta_ts, mva_ts, mvb_ts, lab_ts = {}, {}, {}, {}

        def load_first(n):
            # only what chunk (n, 0) needs: weights for mc=0 + mov_a
            sta_ts[(n, 0)] = loads.tile([128, NSTA, 128], fp8, tag="sta0",
                                        name=f"sta_{n}_0")
            nc.sync.dma_start(
                sta_ts[(n, 0)][:],
                sta_d[n, 0].rearrange("p (s w) -> p s w", s=NSTA),
            )
            mva_ts[n] = loads.tile([128, NMOVA, W], fp8, tag="mva", name=f"mva_{n}")
            nc.sync.dma_start(
                mva_ts[n][:], mva_d[n].rearrange("p (s w) -> p s w", s=NMOVA)
            )

        def load_rest(n):
            for mc in (1, 2):
                sta_ts[(n, mc)] = loads.tile([128, NSTA, 128], fp8,
                                             tag=f"sta{mc}", name=f"sta_{n}_{mc}")
                nc.sync.dma_start(
                    sta_ts[(n, mc)][:],
                    sta_d[n, mc].rearrange("p (s w) -> p s w", s=NSTA),
                )
            mvb_ts[n] = loads.tile([128, NMOVB, W], fp8, tag="mvb", name=f"mvb_{n}")
            nc.sync.dma_start(
                mvb_ts[n][:], mvb_d[n].rearrange("p (s w) -> p s w", s=NMOVB)
            )
            lab_ts[n] = loads.tile([128, MC, W], bf16, tag="lab", name=f"lab_{n}")
            nc.gpsimd.dma_start(
                lab_ts[n][:], lab_d[n].rearrange("p (s w) -> p s w", s=MC)
            )

        load_first(0)
        load_rest(0)

        S_t, D_t = {}, {}

        def stage_mm(i):
            n, mc = i // MC, i % MC
            sta_t = sta_ts[(n, mc)]
            S = spool.tile([128, C, W], bf16, tag="S", name=f"S_{i}")
            S_t[i] = S

            def class_mms(c, out_ap):
                nc.tensor.matmul(
                    out_ap,
                    sta_t[:, 3 * c : 3 * c + 2, :],
                    mva_ts[n][:, 4 * c : 4 * c + 2, :],
                    start=True, stop=False, perf_mode=DR,
                )
                wk = sta_t[:, 3 * c + 2, :]
                w_ap = bass.AP(
                    tensor=wk.tensor, offset=wk.offset,
                    ap=[list(wk.ap[0]), [(NSTA - 1 - (3 * c + 2)) * 128, 2],
                        list(wk.ap[1])],
                )
                if mc == 0:
                    m_ap = mva_ts[n][:, 4 * c + 2 : 4 * c + 4, :]
                elif mc == 1:
                    m_ap = mvb_ts[n][:, 3 * c : 3 * c + 2, :]
                else:
                    mv = mvb_ts[n][:, 3 * c, :]
                    m_ap = bass.AP(
                        tensor=mv.tensor, offset=mv.offset,
                        ap=[list(mv.ap[0]), [2 * W, 2], list(mv.ap[1])],
                    )
                nc.tensor.matmul(
                    out_ap, w_ap, m_ap, start=False, stop=True, perf_mode=DR,
                )

            # three 3-class packs + one 2-class pack per chunk
            for g, (lo, k) in enumerate([(0, 3), (3, 3), (6, 3), (9, 2)]):
                if k == 3:
                    pp = ps3.tile([128, 3, 512], f32, tag="pp", name=f"pp_{i}_{g}")
                else:
                    pp = ps2.tile([128, 2, 512], f32, tag="p2", name=f"p2_{i}")
                for j in range(k):
                    class_mms(lo + j, pp[:, j, 0:W])
                nc.scalar.copy(S[:, lo : lo + k, :], pp[:, 0:k, 0:W])

        def stage_maxsub(i):
            S = S_t[i]
            m5 = tree.tile([128, 5, W], bf16, tag="m5", name=f"m5_{i}")
            nc.vector.tensor_max(m5[:], S[:, 0:5, :], S[:, 5:10, :])
            m2 = tree.tile([128, 2, W], bf16, tag="m2", name=f"m2_{i}")
            nc.vector.tensor_max(m2[:], m5[:, 0:2, :], m5[:, 2:4, :])
            m = small.tile([128, W], bf16, tag="m", name=f"m_{i}")
            nc.vector.tensor_max(m[:], m2[:, 0, :], m2[:, 1, :])
            nc.vector.tensor_max(m[:], m[:], m5[:, 4, :])
            nc.vector.tensor_max(m[:], m[:], S[:, 10, :])

            D = dpool.tile([128, C, W], bf16, tag="D", name=f"D_{i}")
            D_t[i] = D
            m_ap = m[:]
            m_b = bass.AP(
                tensor=m_ap.tensor, offset=m_ap.offset,
                ap=[list(m_ap.ap[0]), [0, C], list(m_ap.ap[1])],
            )
            nc.vector.tensor_sub(D[:], S[:], m_b)

        def stage_exp(i):
            nc.scalar.activation(D_t[i][:], D_t[i][:], AF.Exp)

        def stage_sum(i):
            n, mc = i // MC, i % MC
            D = D_t[i]
            a5 = tree.tile([128, 5, W], bf16, tag="a5", name=f"a5_{i}")
            nc.vector.tensor_add(a5[:], D[:, 0:5, :], D[:, 5:10, :])
            a2 = tree.tile([128, 2, W], bf16, tag="a2", name=f"a2_{i}")
            nc.vector.tensor_add(a2[:], a5[:, 0:2, :], a5[:, 2:4, :])
            acc = small.tile([128, W], bf16, tag="acc", name=f"acc_{i}")
            nc.vector.tensor_add(acc[:], a2[:, 0, :], a2[:, 1, :])
            nc.vector.tensor_add(acc[:], acc[:], a5[:, 4, :])
            nc.vector.tensor_add(acc[:], acc[:], D[:, 10, :])

            # reciprocal: u16 bit-trick seed + one Newton step.
            r0 = small.tile([128, W], bf16, tag="r0", name=f"r0_{i}")
            nc.vector.tensor_sub(
                r0[:].bitcast(u16), magic_t[:], acc[:].bitcast(u16)
            )
            p = small.tile([128, W], bf16, tag="p", name=f"p_{i}")
            nc.vector.tensor_mul(p[:], acc[:], r0[:])
            u = small.tile([128, W], bf16, tag="u", name=f"u_{i}")
            nc.vector.scalar_tensor_tensor(
                out=u[:], in0=p[:], scalar=2.0, in1=r0[:],
                op0=ALU.subtract, op1=ALU.mult,
            )
            w_t = small.tile([128, W], bf16, tag="w", name=f"w_{i}")
            nc.vector.scalar_tensor_tensor(
                out=w_t[:], in0=lab_ts[n][:, mc, :], scalar=BIAS_FIX,
                op0=ALU.mult, op1=ALU.mult, in1=u[:],
                accum_out=partial[:, i : i + 1],
            )

        for i in range(NCH + 3):
            if 0 <= i - 2 < NCH:
                stage_exp(i - 2)
            if i < NCH:
                n = i // MC
                if i % MC == 1 and n + 1 < n_loc:
                    load_first(n + 1)
                if i % MC == 2 and n + 1 < n_loc:
                    load_rest(n + 1)
                stage_mm(i)
            if 0 <= i - 1 < NCH:
                stage_maxsub(i - 1)
            if 0 <= i - 3 < NCH:
                stage_sum(i - 3)

        pf = singles.tile([128, 1], f32)
        nc.vector.tensor_reduce(
            pf[:], partial[:], axis=mybir.AxisListType.X, op=ALU.add
        )
        nc.sync.dma_start(out_d[:, :], pf[:])

    nc.compile()
    return nc


def _get_compiled():
    global _COMPILED
    if _COMPILED is None:
        _COMPILED = _build()
    return _COMPILED


def _host_prep(x, centers, labels):
    x = np.asarray(x, dtype=np.float32)
    centers = np.asarray(centers, dtype=np.float32)
    labels_np = np.asarray(labels)

    n_zero = int((labels_np == 0).sum())

    # sta[n, mc]: [128, NSTA, 128]; slot 3c+kc at [p, q] =
    #   -2*x[n, c, mc*128+q, kc*128+p]; slot 33 = eye(128)
    xt2 = np.transpose(x, (0, 1, 3, 2)) * -2.0          # [N, C, W(k), H]
    xt2 = xt2.reshape(N, C, KC, 128, MC, 128).astype(_FP8)
    sta = np.empty((N, MC, 128, NSTA, 128), dtype=_FP8)
    # -> [N, MC(h), 128(p=k), C, KC, 128(q=h)]
    sta[:, :, :, : C * KC, :] = np.transpose(xt2, (0, 4, 3, 1, 2, 5)).reshape(
        N, MC, 128, C * KC, 128
    )
    sta[:, :, :, C * KC, :] = np.eye(128, dtype=_FP8)[None, None]

    ee = (x * x + centers * centers).astype(_FP8).reshape(N, C, MC, 128, W)
    cc8 = centers.astype(_FP8).reshape(N, C, KC, 128, W)

    mva = np.empty((N, C, 4, 128, W), dtype=_FP8)
    mva[:, :, 0:3] = cc8
    mva[:, :, 3] = ee[:, :, 0]
    mva = np.ascontiguousarray(
        np.transpose(mva, (0, 3, 1, 2, 4)).reshape(N, 128, NMOVA * W)
    )
    mvb = np.empty((N, C, 3, 128, W), dtype=_FP8)
    mvb[:, :, 0] = cc8[:, :, 2]
    mvb[:, :, 1] = ee[:, :, 1]
    mvb[:, :, 2] = ee[:, :, 2]
    mvb = np.ascontiguousarray(
        np.transpose(mvb, (0, 3, 1, 2, 4)).reshape(N, 128, NMOVB * W)
    )

    lab = np.ascontiguousarray(
        np.transpose(
            labels_np.astype(np.float32).reshape(N, MC, 128, W), (0, 2, 1, 3)
        ).reshape(N, 128, MC * W)
    ).astype(_BF16)

    sta = sta.reshape(N, MC, 128, NSTA * 128)
    cst = np.full((128, W), MAGIC, dtype=np.uint16)

    in_maps = []
    for core in range(N_CORES):
        sl = slice(core * N_LOC, (core + 1) * N_LOC)
        in_maps.append(
            {
                "sta": np.ascontiguousarray(sta[sl]),
                "mva": np.ascontiguousarray(mva[sl]),
                "mvb": np.ascontiguousarray(mvb[sl]),
                "lab": np.ascontiguousarray(lab[sl]),
                "cst": cst,
            }
        )
    return in_maps, n_zero


def kernel(x, centers, labels, _trace=False, _trace_kwargs=None):
    from concourse import bass_utils

    nc = _get_compiled()
    in_maps, n_zero = _host_prep(x, centers, labels)

    kwargs = {}
    if _trace:
        kwargs = dict(trace=True, **(_trace_kwargs or {}))
    res = bass_utils.run_bass_kernel_spmd(
        nc, in_maps, core_ids=list(range(N_CORES)), **kwargs
    )

    total = 0.0
    for core in range(N_CORES):
        total += float(res.results[core]["out"].astype(np.float64).sum())
    loss = (total + 1e-12 * n_zero) / float(N * H * W)
    out = np.float32(loss)
    if _trace:
        return out, res
    return out
